# revision 51
# baseline (speedup 1.0000x reference)
"""Trainium2 Bass kernel for RoPE'd causal attention (no softmax).

Reference computation (B=2, H=8, T=2048, N=512, DV=128):
    QR = Q*cos + rotate_half_interleaved(Q)*sin         (K == Q)
    S  = QR @ QR^T          [B,H,T,T]
    S  = tril(S, -1)        (strictly lower triangular)
    O  = S @ V              [B,H,T,DV]

Sharding: the 16 (b,h) pairs are split 2-per-core across 8 NeuronCores.
Each core computes its two T x T score blocks independently; only the
strictly-lower-triangular block tiles are computed (upper tiles skipped),
and diagonal-straddling blocks only compute their live column range.

Device algorithm per (b,h), all-bf16 datapath (fp32 PSUM accumulation):
  - Q / V / cos / sin tables are host-cast to bf16 (halving HBM traffic
    vs fp32; matmul throughput is the same 1 col/cycle but DVE work runs
    at 2x) and host-PRE-TILED so every DMA moves >= 4KB of contiguous
    bytes per partition row: q in whole 4-tile t-groups with its n-axis
    permuted even-pairs-first, cos+sin deduplicated to one column per
    frequency pair and combined into one per-group array, V s-major.
  - RoPE on the vector engine as six dense [P, 256] ops (the even-first
    permutation makes each pair's 2x2 rotation a contiguous slice).
  - QR^T built via PE identity-transposes (~128 cyc each, pipelined;
    the DMA xbar transpose corrupts data under concurrency, measured).
  - Score blocks computed transposed (S^T[s, t-group]) in PSUM, then
    masked (diagonal-straddling) or copied to SBUF as bf16.
  - AV computes O directly (no output transpose): for each 128-wide
    t-chunk c of the group, out[t, d] accumulates
        pso[c][t, d] += st_i[:, c*128:(c+1)*128]^T @ V[i]
    over s-tiles i; chunks with c < (i - 4g) are entirely masked-out and
    skipped.  Completed chunks drain (copy + DMA) one score-block late,
    so the in-order sync queue never head-of-line blocks q-tile loads.
"""

import math

import numpy as np

B, H, T, NDIM, DV = 2, 8, 2048, 512, 128
P = 128            # partitions
NT = T // P        # 16 t-tiles per (b,h)
NG = 4             # t-groups per (b,h)
GW = T // NG       # 512 group width
NK = NDIM // P     # 4 contraction chunks
NH = NDIM // 2     # 256 frequency pairs (cos/sin table width)
NCORES = 8
BH_PER_CORE = (B * H) // NCORES  # 2

TRACE = False          # set by test harness to capture HW profile
LAST_RESULTS = None    # BassKernelResults of the last kernel() call

_NC_CACHE = {}


def _host_tables(freqs):
    """Mirror reference.py's fp32 phase arithmetic exactly, then cast bf16.

    Each frequency pair (2i, 2i+1) shares a phase, so only NDIM/2 cos/sin
    columns are stored; the device RoPE applies the 2x2 rotation per pair.
    """
    import ml_dtypes

    f = np.asarray(freqs, dtype=np.float32).reshape(NDIM)[0::2]  # [256]
    t = np.arange(T, dtype=np.float32)
    ph = t[:, None] * f[None, :]            # fp32 multiply, like jnp
    ph = ph % np.float32(1.0)
    ph = ph * np.float32(2.0 * math.pi)
    bf16 = ml_dtypes.bfloat16
    return np.cos(ph).astype(bf16), np.sin(ph).astype(bf16)


def _emit(tc, nc, aps):
    import concourse.mybir as mybir
    from contextlib import ExitStack
    from concourse.bass import ds, ts

    q, v, cs, o = aps
    f32 = mybir.dt.float32
    bf16 = mybir.dt.bfloat16

    with ExitStack() as ctx:

        def pool(name, bufs, space="SBUF"):
            return ctx.enter_context(
                tc.tile_pool(name=name, bufs=bufs, space=space)
            )

        const = pool("const", 1)
        cospool = pool("cost", NG)
        qin = pool("qin", 4)
        qrp = pool("qr", 6)
        tmpp = pool("tmp", 3)
        qrtp = pool("qrt", 2 * NG)
        stp = pool("st", 4)
        vp = pool("v", 2)
        outp = pool("out", 6)
        ps_tr = pool("pstr", 2, "PSUM")
        ps_s = pool("pss", 2, "PSUM")
        # One open accumulation group per PSUM bank: interleaving the four
        # output-chunk accumulations within a single bank silently drops all
        # but the last-opened chunk's partial sums (measured on HW), so each
        # t-chunk accumulates in its own bank: [P, c, 512-f32-bank].  A
        # single persistent 4-bank tile holds both (b,h): bh0 in columns
        # 0:DV, bh1 in DV:2*DV, so consecutive groups never wait on each
        # other's output drain and each bank only ever sees sequential
        # (never interleaved) accumulation groups.
        ps_o = pool("pso", 1, "PSUM")
        pso_all = ps_o.tile([P, NG, 512], f32, name="pso_all")

        # Constants are built on the otherwise-idle GpSimd engine instead of
        # DMA'd; table DMAs ride the scalar HWDGE ring while q tiles ride
        # the sync ring — the startup is DMA-bound, so every byte and every
        # serialized queue matters.
        ident = const.tile([P, P], f32, name="ident_f32")
        nc.gpsimd.memset(ident[:], 0.0)
        nc.gpsimd.affine_select(
            out=ident[:],
            in_=ident[:],
            compare_op=mybir.AluOpType.not_equal,
            fill=1.0,
            base=0,
            pattern=[[-1, P]],
            channel_multiplier=1,
        )
        ident_b = const.tile([P, P], bf16, name="ident_bf16")
        nc.scalar.copy(ident_b[:], ident[:])

        mask_sb = const.tile([P, NG, GW], f32)
        for d in range(NG):
            # mask_d[sp, tf] = 1.0 iff sp < tf - 128*d
            nc.gpsimd.memset(mask_sb[:, d, :], 1.0)
            nc.gpsimd.affine_select(
                out=mask_sb[:, d, :],
                in_=mask_sb[:, d, :],
                compare_op=mybir.AluOpType.is_ge,
                fill=0.0,
                base=-(P * d + 1),
                pattern=[[1, GW]],
                channel_multiplier=-1,
            )
        # q / cs / v are host-pre-tiled so every DMA moves >=4KB of
        # contiguous bytes per partition row (small per-partition lines
        # throttle DMA packet efficiency): q in whole 4-tile GROUPS,
        # cos+sin combined per group, V s-major.
        cs_t = [None] * NG        # per-group [P, 4(tile), 2(cos/sin), NH]

        def load_group(bh, g, qeng=None):
            """Issue the DMAs for one 4-tile t-group (and its tables)."""
            if bh == 0:
                eng = nc.scalar if g == 0 else nc.sync
                cst = cospool.tile([P, NG, 2, NH], bf16)
                eng.dma_start(cst[:], cs[g])
                cs_t[g] = cst
            qt4 = qin.tile([P, NG, NDIM], bf16)
            (qeng or nc.sync).dma_start(qt4[:], q[bh, g])
            return qt4

        def rope_group(bh, g, qt4):
            """RoPE one 4-tile t-group; returns the four QR tiles."""
            out = []
            for jj in range(NG):
                qt = qt4[:, jj, :]
                qr_tile = qrp.tile([P, NDIM], bf16)
                tmp = tmpp.tile([P, NDIM], bf16)
                # Q arrives with its n-axis permuted even-pairs-first
                # (host-side layout prep; the score contraction is invariant
                # to a global n permutation shared by both operands), so the
                # per-pair 2x2 RoPE rotation is six DENSE [P, 256] ops:
                #   qr_e = qe*c - qo*s ; qr_o = qo*c + qe*s
                qe, qo = qt[:, 0:NH], qt[:, NH:NDIM]
                qre, qro = qr_tile[:, 0:NH], qr_tile[:, NH:NDIM]
                te, to = tmp[:, 0:NH], tmp[:, NH:NDIM]
                cj = cs_t[g][:, jj, 0, :]
                sj = cs_t[g][:, jj, 1, :]
                mul = mybir.AluOpType.mult
                nc.vector.tensor_tensor(qre, qe, cj, mul)
                nc.vector.tensor_tensor(qro, qo, cj, mul)
                nc.vector.tensor_tensor(te, qo, sj, mul)
                nc.vector.tensor_tensor(to, qe, sj, mul)
                nc.vector.tensor_tensor(qre, qre, te, mybir.AluOpType.subtract)
                nc.vector.tensor_tensor(qro, qro, to, mybir.AluOpType.add)
                out.append(qr_tile)
            return out

        def transpose_tile(qrt_g, jj, qr_tile):
            """PE-transpose one RoPE'd t-tile into qrt_g."""
            pst = ps_tr.tile([P, NK, P], bf16)
            for nk in range(NK):
                nc.tensor.transpose(
                    pst[:, nk, :], qr_tile[:, ts(nk, P)], ident_b[:]
                )
            nc.scalar.copy(qrt_g[:, :, ts(jj, P)], pst[:])

        def compute_group(bh, g, qrt_g, qt4):
            qr_tiles = rope_group(bh, g, qt4)
            for jj in range(NG):
                transpose_tile(qrt_g, jj, qr_tiles[jj])

        # For groups > 0 the two (b,h) of this core are interleaved
        # group-by-group: phase A of both, then phase B+C of both, doubling
        # the independent work between pipeline boundaries.  Group 0 instead
        # runs per-bh (phase A then scores immediately) so the first matmuls
        # start as soon as bh0's four t-tiles have landed.
        v_sbs = [
            vp.tile([P, NT, DV], bf16, name=f"v_sb{b_}")
            for b_ in range(BH_PER_CORE)
        ]
        qrt = [[] for _ in range(BH_PER_CORE)]  # [bh][g] QR^T group tiles
        pending_av = None  # previous group's final AV matmuls, deferred
        pending_drains = []  # (bh, g, c, pso) output chunks to copy+DMA

        def flush_drains():
            # Output drains are emitted one emit_bc late: a drain DMA whose
            # copy isn't ready would head-of-line block the in-order sync
            # queue, stalling the NEXT group's q-tile DMAs behind this
            # group's compute.  Deferred, they land after those dispatches.
            for bh_, g_, c_, off_ in pending_drains:
                out_sb = outp.tile([P, DV], f32)
                nc.scalar.copy(out_sb[:], pso_all[:, c_, off_:off_ + DV])
                nc.sync.dma_start(
                    o[bh_, ds(g_ * GW + c_ * P, P), :], out_sb[:]
                )
            pending_drains.clear()

        def emit_bc(bh, g):
            """Phase B+C: score blocks and AV accumulation for one group.

            Diagonal-straddling blocks (d = i - 4g >= 0) are zero for
            t-columns below lo = 128*d, so the score matmuls, the masked
            copy, and the AV matmuls only touch the [lo:GW] column range;
            AV chunks c < d are skipped entirely.
            """
            nonlocal pending_av
            v_sb = v_sbs[bh]
            qrt_g = qrt[bh][g]
            off = bh * DV  # this bh's column region of the shared pso banks
            ns = NG * g + NG  # number of s-tiles for this group
            av_args = []

            def emit_av(i):
                st_i, d_i = av_args[i]
                for c in range(max(d_i, 0), NG):
                    stop = i == NG * g + c
                    nc.tensor.matmul(
                        pso_all[:, c, off:off + DV],
                        st_i[:, ts(c, P)],
                        v_sb[:, i, :],
                        start=(i == 0),
                        stop=stop,
                        skip_group_check=True,
                    )
                    if stop:
                        pending_drains.append((bh, g, c, off))

            for i in range(ns):
                d = i - NG * g
                lo = P * d if d > 0 else 0
                pss = ps_s.tile([P, GW], f32)
                gi, ii = i // NG, i % NG
                for nk in range(NK):
                    nc.tensor.matmul(
                        pss[:, lo:],
                        qrt[bh][gi][:, nk, ts(ii, P)],
                        qrt_g[:, nk, lo:],
                        start=(nk == 0),
                        stop=(nk == NK - 1),
                        skip_group_check=True,
                    )
                st_t = stp.tile([P, GW], bf16)
                if d >= 0:  # diagonal-straddling block: apply mask
                    nc.vector.tensor_tensor(
                        st_t[:, lo:],
                        pss[:, lo:],
                        mask_sb[:, d, lo:],
                        mybir.AluOpType.mult,
                    )
                else:
                    nc.scalar.copy(st_t[:], pss[:])
                av_args.append((st_t, d))
                if i == 0 and pending_av is not None:
                    # previous group's final AV matmuls, deferred past this
                    # group's first scores so their masked copy has finished
                    pending_av()
                    pending_av = None
                # drains present here are >=1 block old (emit_av(i-1) has
                # not yet run), so their copies are ready or nearly so and
                # cannot head-of-line block the sync queue for long.
                flush_drains()
                if i > 0:  # AV matmuls lag one step so the copy can finish
                    emit_av(i - 1)
            pending_av = lambda n_=ns - 1, f_=emit_av: f_(n_)  # noqa: E731

        # Group 0: ALL q/table DMAs are issued before any compute is
        # emitted (so bh1's loads are not queued behind bh0's compute
        # dependencies), V loads after the q tiles, and each bh's scores
        # start as soon as its own four tiles are transposed.
        g0q = {}
        for bh in range(BH_PER_CORE):
            qrt[bh].append(qrtp.tile([P, NK, GW], bf16, name=f"qrt0_{bh}"))
            # bh0's q rides sync, bh1's scalar, so both land in parallel
            g0q[bh] = load_group(bh, 0, qeng=nc.sync if bh == 0 else nc.scalar)
        for bh in range(BH_PER_CORE):
            # V s-tiles arrive just in time (first AV matmuls only read
            # v_sb[:, 0:4]); loading after the q tiles keeps them off the
            # ramp's critical path.
            nc.sync.dma_start(v_sbs[bh][:, 0:NG, :], v[bh][:, 0:NG, :])
        compute_group(0, 0, qrt[0][0], g0q[0])
        emit_bc(0, 0)
        compute_group(1, 0, qrt[1][0], g0q[1])

        # Software-pipelined: group g+1's loads/RoPE/transposes are emitted
        # between bc(bh0, g) and bc(bh1, g), so its transposes run mid-group
        # on the PE and its qrt copies finish during bc(bh1, g) — the next
        # group's first score matmul then starts without the ~0.7us stall
        # of waiting for phase A at the group boundary.
        def phase_a(gn):
            for bh in range(BH_PER_CORE):
                qrt_g = qrtp.tile([P, NK, GW], bf16)
                qrt[bh].append(qrt_g)
                qt4 = load_group(bh, gn)
                compute_group(bh, gn, qrt_g, qt4)
                if gn == 1:
                    nc.sync.dma_start(v_sbs[bh][:, NG:, :], v[bh][:, NG:, :])

        phase_a(1)
        emit_bc(1, 0)
        for g in range(1, NG):
            emit_bc(0, g)
            if g + 1 < NG:
                phase_a(g + 1)
            emit_bc(1, g)
        pending_av()  # final group's last AV matmuls
        flush_drains()  # final group's output chunks


def build_nc():
    import concourse.bass as bass  # noqa: F401
    import concourse.mybir as mybir
    import concourse.tile as tile
    from concourse import bacc

    nc = bacc.Bacc(
        "TRN2",
        target_bir_lowering=False,
        debug=False,
        enable_asserts=False,
        num_devices=NCORES,
    )
    f32 = mybir.dt.float32
    bf16 = mybir.dt.bfloat16
    q = nc.dram_tensor(
        "q", [BH_PER_CORE, NG, P, NG, NDIM], bf16, kind="ExternalInput"
    ).ap()
    v = nc.dram_tensor(
        "v", [BH_PER_CORE, P, NT, DV], bf16, kind="ExternalInput"
    ).ap()
    cs = nc.dram_tensor(
        "cs", [NG, P, NG, 2, NH], bf16, kind="ExternalInput"
    ).ap()
    o = nc.dram_tensor("o", [BH_PER_CORE, T, DV], f32, kind="ExternalOutput").ap()

    with tile.TileContext(nc) as tc:
        _emit(tc, nc, (q, v, cs, o))
    nc.compile()
    return nc


def get_nc():
    if "nc" not in _NC_CACHE:
        _NC_CACHE["nc"] = build_nc()
    return _NC_CACHE["nc"]


def make_in_maps(Q, V, freqs):
    import ml_dtypes

    bf = ml_dtypes.bfloat16
    Q = np.asarray(Q, dtype=np.float32).reshape(B * H, T, NDIM)
    # even-pairs-first n permutation (see phase_a_pair)
    Q = np.concatenate([Q[..., 0::2], Q[..., 1::2]], axis=-1).astype(bf)
    # t-tile groups: [bh, 4, 128, 4, 512] so each q DMA reads 4KB/partition
    Q = Q.reshape(B * H, NG, NG, P, NDIM).transpose(0, 1, 3, 2, 4)
    V = np.asarray(V, dtype=np.float32).reshape(B * H, T, DV).astype(bf)
    # s-major: [bh, s, i, d] so V DMAs read 1KB+/partition
    V = V.reshape(B * H, NT, P, DV).transpose(0, 2, 1, 3)
    cosv, sinv = _host_tables(freqs)  # [T, NH] each
    cs = np.stack([cosv, sinv], axis=1).reshape(NG, NG, P, 2, NH)
    cs = np.ascontiguousarray(cs.transpose(0, 2, 1, 3, 4))  # [4,128,4,2,NH]
    in_maps = []
    for c in range(NCORES):
        in_maps.append(
            {
                "q": np.ascontiguousarray(Q[BH_PER_CORE * c : BH_PER_CORE * (c + 1)]),
                "v": np.ascontiguousarray(V[BH_PER_CORE * c : BH_PER_CORE * (c + 1)]),
                "cs": cs,
            }
        )
    return in_maps


def kernel(Q, V, freqs):
    global LAST_RESULTS
    from concourse.bass_utils import run_bass_kernel_spmd

    nc = get_nc()
    in_maps = make_in_maps(Q, V, freqs)
    res = run_bass_kernel_spmd(
        nc, in_maps, core_ids=list(range(NCORES)), trace=TRACE
    )
    LAST_RESULTS = res
    out = np.stack([r["o"] for r in res.results])  # [8, 2, T, DV]
    return out.reshape(B, H, T, DV).astype(np.float32)


# revision 52
# speedup vs baseline: 1.0434x; 1.0434x over previous
"""Trainium2 Bass kernel for RoPE'd causal attention (no softmax).

Reference computation (B=2, H=8, T=2048, N=512, DV=128):
    QR = Q*cos + rotate_half_interleaved(Q)*sin         (K == Q)
    S  = QR @ QR^T          [B,H,T,T]
    S  = tril(S, -1)        (strictly lower triangular)
    O  = S @ V              [B,H,T,DV]

Sharding: the 16 (b,h) pairs are split 2-per-core across 8 NeuronCores.
Each core computes its two T x T score blocks independently; only the
strictly-lower-triangular block tiles are computed (upper tiles skipped),
and diagonal-straddling blocks only compute their live column range.

Device algorithm per (b,h), all-bf16 datapath (fp32 PSUM accumulation):
  - Q / V / cos / sin tables are host-cast to bf16 (halving HBM traffic
    vs fp32; matmul throughput is the same 1 col/cycle but DVE work runs
    at 2x) and host-PRE-TILED so every DMA moves >= 4KB of contiguous
    bytes per partition row: q in whole 4-tile t-groups with its n-axis
    permuted even-pairs-first, cos+sin deduplicated to one column per
    frequency pair and combined into one per-group array, V s-major.
  - RoPE on the vector engine as six dense [P, 256] ops (the even-first
    permutation makes each pair's 2x2 rotation a contiguous slice).
  - QR^T built via PE identity-transposes (~128 cyc each, pipelined;
    the DMA xbar transpose corrupts data under concurrency, measured).
  - Score blocks computed transposed (S^T[s, t-group]) in PSUM, then
    masked (diagonal-straddling) or copied to SBUF as bf16.
  - AV computes O directly (no output transpose): for each 128-wide
    t-chunk c of the group, out[t, d] accumulates
        pso[c][t, d] += st_i[:, c*128:(c+1)*128]^T @ V[i]
    over s-tiles i; chunks with c < (i - 4g) are entirely masked-out and
    skipped.  Completed chunks drain (copy + DMA) one score-block late,
    so the in-order sync queue never head-of-line blocks q-tile loads.
"""

import math

import numpy as np

B, H, T, NDIM, DV = 2, 8, 2048, 512, 128
P = 128            # partitions
NT = T // P        # 16 t-tiles per (b,h)
NG = 4             # t-groups per (b,h)
GW = T // NG       # 512 group width
NK = NDIM // P     # 4 contraction chunks
NH = NDIM // 2     # 256 frequency pairs (cos/sin table width)
NCORES = 8
BH_PER_CORE = (B * H) // NCORES  # 2

TRACE = False          # set by test harness to capture HW profile
LAST_RESULTS = None    # BassKernelResults of the last kernel() call

_NC_CACHE = {}


def _host_tables(freqs):
    """Mirror reference.py's fp32 phase arithmetic exactly, then cast bf16.

    Each frequency pair (2i, 2i+1) shares a phase, so only NDIM/2 cos/sin
    columns are stored; the device RoPE applies the 2x2 rotation per pair.
    """
    import ml_dtypes

    f = np.asarray(freqs, dtype=np.float32).reshape(NDIM)[0::2]  # [256]
    t = np.arange(T, dtype=np.float32)
    ph = t[:, None] * f[None, :]            # fp32 multiply, like jnp
    ph = ph % np.float32(1.0)
    ph = ph * np.float32(2.0 * math.pi)
    bf16 = ml_dtypes.bfloat16
    return np.cos(ph).astype(bf16), np.sin(ph).astype(bf16)


def _emit(tc, nc, aps):
    import concourse.mybir as mybir
    from contextlib import ExitStack
    from concourse.bass import ds, ts

    q, v, cs, o = aps
    f32 = mybir.dt.float32
    bf16 = mybir.dt.bfloat16

    with ExitStack() as ctx:

        def pool(name, bufs, space="SBUF"):
            return ctx.enter_context(
                tc.tile_pool(name=name, bufs=bufs, space=space)
            )

        const = pool("const", 1)
        cospool = pool("cost", NG)
        qin = pool("qin", 4)
        qrp = pool("qr", 6)
        tmpp = pool("tmp", 3)
        qrtp = pool("qrt", 2 * NG)
        stp = pool("st", 4)
        vp = pool("v", 2)
        outp = pool("out", 6)
        ps_tr = pool("pstr", 2, "PSUM")
        ps_s = pool("pss", 2, "PSUM")
        # One open accumulation group per PSUM bank: interleaving the four
        # output-chunk accumulations within a single bank silently drops all
        # but the last-opened chunk's partial sums (measured on HW), so each
        # t-chunk accumulates in its own bank: [P, c, 512-f32-bank].  A
        # single persistent 4-bank tile holds both (b,h): bh0 in columns
        # 0:DV, bh1 in DV:2*DV, so consecutive groups never wait on each
        # other's output drain and each bank only ever sees sequential
        # (never interleaved) accumulation groups.
        ps_o = pool("pso", 1, "PSUM")
        pso_all = ps_o.tile([P, NG, 512], f32, name="pso_all")

        # Constants are built on the otherwise-idle GpSimd engine instead of
        # DMA'd; table DMAs ride the scalar HWDGE ring while q tiles ride
        # the sync ring — the startup is DMA-bound, so every byte and every
        # serialized queue matters.
        ident = const.tile([P, P], f32, name="ident_f32")
        nc.gpsimd.memset(ident[:], 0.0)
        nc.gpsimd.affine_select(
            out=ident[:],
            in_=ident[:],
            compare_op=mybir.AluOpType.not_equal,
            fill=1.0,
            base=0,
            pattern=[[-1, P]],
            channel_multiplier=1,
        )
        ident_b = const.tile([P, P], bf16, name="ident_bf16")
        nc.scalar.copy(ident_b[:], ident[:])

        mask_sb = const.tile([P, NG, GW], f32)
        for d in range(NG):
            # mask_d[sp, tf] = 1.0 iff sp < tf - 128*d
            nc.gpsimd.memset(mask_sb[:, d, :], 1.0)
            nc.gpsimd.affine_select(
                out=mask_sb[:, d, :],
                in_=mask_sb[:, d, :],
                compare_op=mybir.AluOpType.is_ge,
                fill=0.0,
                base=-(P * d + 1),
                pattern=[[1, GW]],
                channel_multiplier=-1,
            )
        # q / cs / v are host-pre-tiled so every DMA moves >=4KB of
        # contiguous bytes per partition row (small per-partition lines
        # throttle DMA packet efficiency): q in whole 4-tile GROUPS,
        # cos+sin combined per group, V s-major.
        cs_t = [None] * NG        # per-group [P, 4(tile), 2(cos/sin), NH]

        def load_group(bh, g, qeng=None):
            """Issue the DMAs for one 4-tile t-group (and its tables)."""
            if bh == 0:
                eng = nc.scalar if g == 0 else nc.sync
                cst = cospool.tile([P, NG, 2, NH], bf16)
                eng.dma_start(cst[:], cs[g])
                cs_t[g] = cst
            qt4 = qin.tile([P, NG, NDIM], bf16)
            (qeng or nc.sync).dma_start(qt4[:], q[bh, g])
            return qt4

        def rope_group(bh, g, qt4):
            """RoPE one 4-tile t-group; returns the four QR tiles."""
            out = []
            for jj in range(NG):
                qt = qt4[:, jj, :]
                qr_tile = qrp.tile([P, NDIM], bf16)
                tmp = tmpp.tile([P, NDIM], bf16)
                # Q arrives with its n-axis permuted even-pairs-first
                # (host-side layout prep; the score contraction is invariant
                # to a global n permutation shared by both operands), so the
                # per-pair 2x2 RoPE rotation is six DENSE [P, 256] ops:
                #   qr_e = qe*c - qo*s ; qr_o = qo*c + qe*s
                qe, qo = qt[:, 0:NH], qt[:, NH:NDIM]
                qre, qro = qr_tile[:, 0:NH], qr_tile[:, NH:NDIM]
                te, to = tmp[:, 0:NH], tmp[:, NH:NDIM]
                cj = cs_t[g][:, jj, 0, :]
                sj = cs_t[g][:, jj, 1, :]
                mul = mybir.AluOpType.mult
                nc.vector.tensor_tensor(qre, qe, cj, mul)
                nc.vector.tensor_tensor(qro, qo, cj, mul)
                nc.vector.tensor_tensor(te, qo, sj, mul)
                nc.vector.tensor_tensor(to, qe, sj, mul)
                nc.vector.tensor_tensor(qre, qre, te, mybir.AluOpType.subtract)
                nc.vector.tensor_tensor(qro, qro, to, mybir.AluOpType.add)
                out.append(qr_tile)
            return out

        def transpose_tile(qrt_g, jj, qr_tile):
            """PE-transpose one RoPE'd t-tile into qrt_g."""
            pst = ps_tr.tile([P, NK, P], bf16)
            for nk in range(NK):
                nc.tensor.transpose(
                    pst[:, nk, :], qr_tile[:, ts(nk, P)], ident_b[:]
                )
            nc.scalar.copy(qrt_g[:, :, ts(jj, P)], pst[:])

        def compute_group(bh, g, qrt_g, qt4):
            qr_tiles = rope_group(bh, g, qt4)
            for jj in range(NG):
                transpose_tile(qrt_g, jj, qr_tiles[jj])

        # For groups > 0 the two (b,h) of this core are interleaved
        # group-by-group: phase A of both, then phase B+C of both, doubling
        # the independent work between pipeline boundaries.  Group 0 instead
        # runs per-bh (phase A then scores immediately) so the first matmuls
        # start as soon as bh0's four t-tiles have landed.
        v_sbs = [
            vp.tile([P, NT, DV], bf16, name=f"v_sb{b_}")
            for b_ in range(BH_PER_CORE)
        ]
        qrt = [[] for _ in range(BH_PER_CORE)]  # [bh][g] QR^T group tiles
        pending_av = None  # previous group's final AV matmuls, deferred
        pending_drains = []  # (bh, g, c, pso) output chunks to copy+DMA

        def flush_drains():
            # Output drains are emitted one emit_bc late: a drain DMA whose
            # copy isn't ready would head-of-line block the in-order sync
            # queue, stalling the NEXT group's q-tile DMAs behind this
            # group's compute.  Deferred, they land after those dispatches.
            for bh_, g_, c_, off_ in pending_drains:
                out_sb = outp.tile([P, DV], f32)
                nc.scalar.copy(out_sb[:], pso_all[:, c_, off_:off_ + DV])
                nc.sync.dma_start(
                    o[bh_, ds(g_ * GW + c_ * P, P), :], out_sb[:]
                )
            pending_drains.clear()

        def emit_bc(bh, g):
            """Phase B+C: score blocks and AV accumulation for one group.

            Diagonal-straddling blocks (d = i - 4g >= 0) are zero for
            t-columns below lo = 128*d, so the score matmuls, the masked
            copy, and the AV matmuls only touch the [lo:GW] column range;
            AV chunks c < d are skipped entirely.
            """
            nonlocal pending_av
            v_sb = v_sbs[bh]
            qrt_g = qrt[bh][g]
            off = bh * DV  # this bh's column region of the shared pso banks
            ns = NG * g + NG  # number of s-tiles for this group
            av_args = []

            def emit_av(i):
                st_i, d_i = av_args[i]
                for c in range(max(d_i, 0), NG):
                    stop = i == NG * g + c
                    nc.tensor.matmul(
                        pso_all[:, c, off:off + DV],
                        st_i[:, ts(c, P)],
                        v_sb[:, i, :],
                        start=(i == 0),
                        stop=stop,
                        skip_group_check=True,
                    )
                    if stop:
                        pending_drains.append((bh, g, c, off))

            for i in range(ns):
                d = i - NG * g
                lo = P * d if d > 0 else 0
                pss = ps_s.tile([P, GW], f32)
                gi, ii = i // NG, i % NG
                for nk in range(NK):
                    nc.tensor.matmul(
                        pss[:, lo:],
                        qrt[bh][gi][:, nk, ts(ii, P)],
                        qrt_g[:, nk, lo:],
                        start=(nk == 0),
                        stop=(nk == NK - 1),
                        skip_group_check=True,
                    )
                st_t = stp.tile([P, GW], bf16)
                if d >= 0:  # diagonal-straddling block: apply mask
                    nc.vector.tensor_tensor(
                        st_t[:, lo:],
                        pss[:, lo:],
                        mask_sb[:, d, lo:],
                        mybir.AluOpType.mult,
                    )
                else:
                    nc.scalar.copy(st_t[:], pss[:])
                av_args.append((st_t, d))
                if i == 0 and pending_av is not None:
                    # previous group's final AV matmuls, deferred past this
                    # group's first scores so their masked copy has finished
                    pending_av()
                    pending_av = None
                # drains present here are >=1 block old (emit_av(i-1) has
                # not yet run), so their copies are ready or nearly so and
                # cannot head-of-line block the sync queue for long.
                flush_drains()
                if i > 0:  # AV matmuls lag one step so the copy can finish
                    emit_av(i - 1)
            pending_av = lambda n_=ns - 1, f_=emit_av: f_(n_)  # noqa: E731

        # Group 0: ALL q/table DMAs are issued before any compute is
        # emitted (so bh1's loads are not queued behind bh0's compute
        # dependencies), V loads after the q tiles, and each bh's scores
        # start as soon as its own four tiles are transposed.
        g0q = {}
        for bh in range(BH_PER_CORE):
            qrt[bh].append(qrtp.tile([P, NK, GW], bf16, name=f"qrt0_{bh}"))
            # bh0's q rides sync, bh1's scalar, so both land in parallel
            g0q[bh] = load_group(bh, 0, qeng=nc.sync if bh == 0 else nc.scalar)
        for bh in range(BH_PER_CORE):
            # V s-tiles arrive just in time (first AV matmuls only read
            # v_sb[:, 0:4]); loading after the q tiles keeps them off the
            # ramp's critical path.
            nc.sync.dma_start(v_sbs[bh][:, 0:NG, :], v[bh][:, 0:NG, :])
        for bh in range(BH_PER_CORE):
            compute_group(bh, 0, qrt[bh][0], g0q[bh])
            emit_bc(bh, 0)

        for g in range(1, NG):
            for bh in range(BH_PER_CORE):
                qrt_g = qrtp.tile([P, NK, GW], bf16)
                qrt[bh].append(qrt_g)
                qt4 = load_group(bh, g)
                compute_group(bh, g, qrt_g, qt4)
                if g == 1:
                    nc.sync.dma_start(v_sbs[bh][:, NG:, :], v[bh][:, NG:, :])
            for bh in range(BH_PER_CORE):
                emit_bc(bh, g)
        pending_av()  # final group's last AV matmuls
        flush_drains()  # final group's output chunks


def build_nc():
    import concourse.bass as bass  # noqa: F401
    import concourse.mybir as mybir
    import concourse.tile as tile
    from concourse import bacc

    nc = bacc.Bacc(
        "TRN2",
        target_bir_lowering=False,
        debug=False,
        enable_asserts=False,
        num_devices=NCORES,
    )
    f32 = mybir.dt.float32
    bf16 = mybir.dt.bfloat16
    q = nc.dram_tensor(
        "q", [BH_PER_CORE, NG, P, NG, NDIM], bf16, kind="ExternalInput"
    ).ap()
    v = nc.dram_tensor(
        "v", [BH_PER_CORE, P, NT, DV], bf16, kind="ExternalInput"
    ).ap()
    cs = nc.dram_tensor(
        "cs", [NG, P, NG, 2, NH], bf16, kind="ExternalInput"
    ).ap()
    o = nc.dram_tensor("o", [BH_PER_CORE, T, DV], f32, kind="ExternalOutput").ap()

    with tile.TileContext(nc) as tc:
        _emit(tc, nc, (q, v, cs, o))
    nc.compile()
    return nc


def get_nc():
    if "nc" not in _NC_CACHE:
        _NC_CACHE["nc"] = build_nc()
    return _NC_CACHE["nc"]


def make_in_maps(Q, V, freqs):
    import ml_dtypes

    bf = ml_dtypes.bfloat16
    Q = np.asarray(Q, dtype=np.float32).reshape(B * H, T, NDIM)
    # even-pairs-first n permutation (see phase_a_pair)
    Q = np.concatenate([Q[..., 0::2], Q[..., 1::2]], axis=-1).astype(bf)
    # t-tile groups: [bh, 4, 128, 4, 512] so each q DMA reads 4KB/partition
    Q = Q.reshape(B * H, NG, NG, P, NDIM).transpose(0, 1, 3, 2, 4)
    V = np.asarray(V, dtype=np.float32).reshape(B * H, T, DV).astype(bf)
    # s-major: [bh, s, i, d] so V DMAs read 1KB+/partition
    V = V.reshape(B * H, NT, P, DV).transpose(0, 2, 1, 3)
    cosv, sinv = _host_tables(freqs)  # [T, NH] each
    cs = np.stack([cosv, sinv], axis=1).reshape(NG, NG, P, 2, NH)
    cs = np.ascontiguousarray(cs.transpose(0, 2, 1, 3, 4))  # [4,128,4,2,NH]
    in_maps = []
    for c in range(NCORES):
        in_maps.append(
            {
                "q": np.ascontiguousarray(Q[BH_PER_CORE * c : BH_PER_CORE * (c + 1)]),
                "v": np.ascontiguousarray(V[BH_PER_CORE * c : BH_PER_CORE * (c + 1)]),
                "cs": cs,
            }
        )
    return in_maps


def kernel(Q, V, freqs):
    global LAST_RESULTS
    from concourse.bass_utils import run_bass_kernel_spmd

    nc = get_nc()
    in_maps = make_in_maps(Q, V, freqs)
    res = run_bass_kernel_spmd(
        nc, in_maps, core_ids=list(range(NCORES)), trace=TRACE
    )
    LAST_RESULTS = res
    out = np.stack([r["o"] for r in res.results])  # [8, 2, T, DV]
    return out.reshape(B, H, T, DV).astype(np.float32)


# revision 53
# speedup vs baseline: 1.2379x; 1.1865x over previous
"""Trainium2 Bass kernel for RoPE'd causal attention (no softmax).

Reference computation (B=2, H=8, T=2048, N=512, DV=128):
    QR = Q*cos + rotate_half_interleaved(Q)*sin         (K == Q)
    S  = QR @ QR^T          [B,H,T,T]
    S  = tril(S, -1)        (strictly lower triangular)
    O  = S @ V              [B,H,T,DV]

Sharding: the 16 (b,h) pairs are split 2-per-core across 8 NeuronCores.
Each core computes its two T x T score blocks independently; only the
strictly-lower-triangular block tiles are computed (upper tiles skipped),
and diagonal-straddling blocks only compute their live column range.

Device algorithm per (b,h), all-bf16 datapath (fp32 PSUM accumulation):
  - Q / V / cos / sin tables are host-cast to bf16 (halving HBM traffic
    vs fp32; matmul throughput is the same 1 col/cycle but DVE work runs
    at 2x) and host-PRE-TILED so every DMA moves >= 4KB of contiguous
    bytes per partition row: q in whole 4-tile t-groups with its n-axis
    permuted even-pairs-first, cos+sin deduplicated to one column per
    frequency pair and combined into one per-group array, V s-major.
  - RoPE on the vector engine as six dense [P, 256] ops (the even-first
    permutation makes each pair's 2x2 rotation a contiguous slice).
  - QR^T built via PE identity-transposes (~128 cyc each, pipelined;
    the DMA xbar transpose corrupts data under concurrency, measured).
  - Score blocks computed transposed (S^T[s, t-group]) in PSUM, then
    masked (diagonal-straddling) or copied to SBUF as bf16.
  - AV computes O directly (no output transpose): for each 128-wide
    t-chunk c of the group, out[t, d] accumulates
        pso[c][t, d] += st_i[:, c*128:(c+1)*128]^T @ V[i]
    over s-tiles i; chunks with c < (i - 4g) are entirely masked-out and
    skipped.  Completed chunks drain (copy + DMA) one score-block late,
    so the in-order sync queue never head-of-line blocks q-tile loads.
"""

import math

import numpy as np

B, H, T, NDIM, DV = 2, 8, 2048, 512, 128
P = 128            # partitions
NT = T // P        # 16 t-tiles per (b,h)
NG = 4             # t-groups per (b,h)
GW = T // NG       # 512 group width
NK = NDIM // P     # 4 contraction chunks
NH = NDIM // 2     # 256 frequency pairs (cos/sin table width)
NCORES = 8
BH_PER_CORE = (B * H) // NCORES  # 2

TRACE = False          # set by test harness to capture HW profile
LAST_RESULTS = None    # BassKernelResults of the last kernel() call

_NC_CACHE = {}


def _host_tables(freqs):
    """Mirror reference.py's fp32 phase arithmetic exactly, then cast bf16.

    Each frequency pair (2i, 2i+1) shares a phase, so only NDIM/2 cos/sin
    columns are stored; the device RoPE applies the 2x2 rotation per pair.
    """
    import ml_dtypes

    f = np.asarray(freqs, dtype=np.float32).reshape(NDIM)[0::2]  # [256]
    t = np.arange(T, dtype=np.float32)
    ph = t[:, None] * f[None, :]            # fp32 multiply, like jnp
    ph = ph % np.float32(1.0)
    ph = ph * np.float32(2.0 * math.pi)
    bf16 = ml_dtypes.bfloat16
    return np.cos(ph).astype(bf16), np.sin(ph).astype(bf16)


def _emit(tc, nc, aps):
    import concourse.mybir as mybir
    from contextlib import ExitStack
    from concourse.bass import ds, ts

    q, v, cs, o = aps
    f32 = mybir.dt.float32
    bf16 = mybir.dt.bfloat16

    with ExitStack() as ctx:

        def pool(name, bufs, space="SBUF"):
            return ctx.enter_context(
                tc.tile_pool(name=name, bufs=bufs, space=space)
            )

        const = pool("const", 1)
        cospool = pool("cost", NG)
        qin = pool("qin", 3)
        qrp = pool("qr", 3)
        tmpp = pool("tmp", 3)
        qrtp = pool("qrt", 2 * NG)
        stp = pool("st", 4)
        vp = pool("v", 2)
        outp = pool("out", 6)
        ps_tr = pool("pstr", 2, "PSUM")
        ps_s = pool("pss", 2, "PSUM")
        # One open accumulation group per PSUM bank: interleaving the four
        # output-chunk accumulations within a single bank silently drops all
        # but the last-opened chunk's partial sums (measured on HW), so each
        # t-chunk accumulates in its own bank: [P, c, 512-f32-bank].  A
        # single persistent 4-bank tile holds both (b,h): bh0 in columns
        # 0:DV, bh1 in DV:2*DV, so consecutive groups never wait on each
        # other's output drain and each bank only ever sees sequential
        # (never interleaved) accumulation groups.
        ps_o = pool("pso", 1, "PSUM")
        pso_all = ps_o.tile([P, NG, 512], f32, name="pso_all")

        # Constants are built on the otherwise-idle GpSimd engine instead of
        # DMA'd; table DMAs ride the scalar HWDGE ring while q tiles ride
        # the sync ring — the startup is DMA-bound, so every byte and every
        # serialized queue matters.
        ident = const.tile([P, P], f32, name="ident_f32")
        nc.gpsimd.memset(ident[:], 0.0)
        nc.gpsimd.affine_select(
            out=ident[:],
            in_=ident[:],
            compare_op=mybir.AluOpType.not_equal,
            fill=1.0,
            base=0,
            pattern=[[-1, P]],
            channel_multiplier=1,
        )
        ident_b = const.tile([P, P], bf16, name="ident_bf16")
        nc.scalar.copy(ident_b[:], ident[:])

        mask_sb = const.tile([P, NG, GW], f32)
        for d in range(NG):
            # mask_d[sp, tf] = 1.0 iff sp < tf - 128*d
            nc.gpsimd.memset(mask_sb[:, d, :], 1.0)
            nc.gpsimd.affine_select(
                out=mask_sb[:, d, :],
                in_=mask_sb[:, d, :],
                compare_op=mybir.AluOpType.is_ge,
                fill=0.0,
                base=-(P * d + 1),
                pattern=[[1, GW]],
                channel_multiplier=-1,
            )
        # q / cs / v are host-pre-tiled so every DMA moves >=4KB of
        # contiguous bytes per partition row (small per-partition lines
        # throttle DMA packet efficiency): q in whole 4-tile GROUPS,
        # cos+sin combined per group, V s-major.
        cs_t = [None] * NG        # per-group [P, 4(tile), 2(cos/sin), NH]

        def load_group(bh, g, qeng=None):
            """Issue the DMAs for one 4-tile t-group (and its tables)."""
            if bh == 0:
                eng = nc.scalar if g == 0 else nc.sync
                cst = cospool.tile([P, NG, 2, NH], bf16)
                eng.dma_start(cst[:], cs[g])
                cs_t[g] = cst
            qt4 = qin.tile([P, NG, NDIM], bf16)
            (qeng or nc.sync).dma_start(qt4[:], q[bh, g])
            return qt4

        def rope_group(bh, g, qt4):
            """RoPE one 4-tile t-group; returns the four QR tiles."""
            out = []
            for jj in range(NG):
                qt = qt4[:, jj, :]
                qr_tile = qrp.tile([P, NDIM], bf16)
                tmp = tmpp.tile([P, NDIM], bf16)
                # Q arrives with its n-axis permuted even-pairs-first
                # (host-side layout prep; the score contraction is invariant
                # to a global n permutation shared by both operands), so the
                # per-pair 2x2 RoPE rotation is six DENSE [P, 256] ops:
                #   qr_e = qe*c - qo*s ; qr_o = qo*c + qe*s
                qe, qo = qt[:, 0:NH], qt[:, NH:NDIM]
                qre, qro = qr_tile[:, 0:NH], qr_tile[:, NH:NDIM]
                te, to = tmp[:, 0:NH], tmp[:, NH:NDIM]
                cj = cs_t[g][:, jj, 0, :]
                sj = cs_t[g][:, jj, 1, :]
                mul = mybir.AluOpType.mult
                nc.vector.tensor_tensor(qre, qe, cj, mul)
                nc.vector.tensor_tensor(qro, qo, cj, mul)
                nc.vector.tensor_tensor(te, qo, sj, mul)
                nc.vector.tensor_tensor(to, qe, sj, mul)
                nc.vector.tensor_tensor(qre, qre, te, mybir.AluOpType.subtract)
                nc.vector.tensor_tensor(qro, qro, to, mybir.AluOpType.add)
                out.append(qr_tile)
            return out

        def transpose_tile(qrt_g, jj, qr_tile):
            """PE-transpose one RoPE'd t-tile into qrt_g."""
            pst = ps_tr.tile([P, NK, P], bf16)
            for nk in range(NK):
                nc.tensor.transpose(
                    pst[:, nk, :], qr_tile[:, ts(nk, P)], ident_b[:]
                )
            nc.scalar.copy(qrt_g[:, :, ts(jj, P)], pst[:])

        def compute_group(bh, g, qrt_g, qt4):
            qr_tiles = rope_group(bh, g, qt4)
            for jj in range(NG):
                transpose_tile(qrt_g, jj, qr_tiles[jj])

        # For groups > 0 the two (b,h) of this core are interleaved
        # group-by-group: phase A of both, then phase B+C of both, doubling
        # the independent work between pipeline boundaries.  Group 0 instead
        # runs per-bh (phase A then scores immediately) so the first matmuls
        # start as soon as bh0's four t-tiles have landed.
        v_sbs = [
            vp.tile([P, NT, DV], bf16, name=f"v_sb{b_}")
            for b_ in range(BH_PER_CORE)
        ]
        qrt = [[] for _ in range(BH_PER_CORE)]  # [bh][g] QR^T group tiles
        pending_av = None  # previous group's final AV matmuls, deferred
        pending_drains = []  # (bh, g, c, pso) output chunks to copy+DMA

        def flush_drains():
            # Output drains are emitted one emit_bc late: a drain DMA whose
            # copy isn't ready would head-of-line block the in-order sync
            # queue, stalling the NEXT group's q-tile DMAs behind this
            # group's compute.  Deferred, they land after those dispatches.
            for bh_, g_, c_, off_ in pending_drains:
                out_sb = outp.tile([P, DV], f32)
                nc.scalar.copy(out_sb[:], pso_all[:, c_, off_:off_ + DV])
                nc.sync.dma_start(
                    o[bh_, ds(g_ * GW + c_ * P, P), :], out_sb[:]
                )
            pending_drains.clear()

        def emit_bc(bh, g):
            """Phase B+C: score blocks and AV accumulation for one group.

            Diagonal-straddling blocks (d = i - 4g >= 0) are zero for
            t-columns below lo = 128*d, so the score matmuls, the masked
            copy, and the AV matmuls only touch the [lo:GW] column range;
            AV chunks c < d are skipped entirely.
            """
            nonlocal pending_av
            v_sb = v_sbs[bh]
            qrt_g = qrt[bh][g]
            off = bh * DV  # this bh's column region of the shared pso banks
            ns = NG * g + NG  # number of s-tiles for this group
            av_args = []

            def emit_av(i):
                st_i, d_i = av_args[i]
                for c in range(max(d_i, 0), NG):
                    stop = i == NG * g + c
                    nc.tensor.matmul(
                        pso_all[:, c, off:off + DV],
                        st_i[:, ts(c, P)],
                        v_sb[:, i, :],
                        start=(i == 0),
                        stop=stop,
                        skip_group_check=True,
                    )
                    if stop:
                        pending_drains.append((bh, g, c, off))

            for i in range(ns):
                d = i - NG * g
                lo = P * d if d > 0 else 0
                pss = ps_s.tile([P, GW], f32)
                gi, ii = i // NG, i % NG
                for nk in range(NK):
                    nc.tensor.matmul(
                        pss[:, lo:],
                        qrt[bh][gi][:, nk, ts(ii, P)],
                        qrt_g[:, nk, lo:],
                        start=(nk == 0),
                        stop=(nk == NK - 1),
                        skip_group_check=True,
                    )
                st_t = stp.tile([P, GW], bf16)
                if d >= 0:  # diagonal-straddling block: apply mask
                    nc.vector.tensor_tensor(
                        st_t[:, lo:],
                        pss[:, lo:],
                        mask_sb[:, d, lo:],
                        mybir.AluOpType.mult,
                    )
                else:
                    nc.scalar.copy(st_t[:], pss[:])
                av_args.append((st_t, d))
                if i == 0 and pending_av is not None:
                    # previous group's final AV matmuls, deferred past this
                    # group's first scores so their masked copy has finished
                    pending_av()
                    pending_av = None
                # drains present here are >=1 block old (emit_av(i-1) has
                # not yet run), so their copies are ready or nearly so and
                # cannot head-of-line block the sync queue for long.
                flush_drains()
                if i > 0:  # AV matmuls lag one step so the copy can finish
                    emit_av(i - 1)
            pending_av = lambda n_=ns - 1, f_=emit_av: f_(n_)  # noqa: E731

        # Group 0: ALL q/table DMAs are issued before any compute is
        # emitted (so bh1's loads are not queued behind bh0's compute
        # dependencies), V loads after the q tiles, and each bh's scores
        # start as soon as its own four tiles are transposed.
        g0q = {}
        for bh in range(BH_PER_CORE):
            qrt[bh].append(qrtp.tile([P, NK, GW], bf16, name=f"qrt0_{bh}"))
            # bh0's q rides sync, bh1's scalar, so both land in parallel
            g0q[bh] = load_group(bh, 0, qeng=nc.sync if bh == 0 else nc.scalar)
        for bh in range(BH_PER_CORE):
            # V s-tiles arrive just in time (first AV matmuls only read
            # v_sb[:, 0:4]); loading after the q tiles keeps them off the
            # ramp's critical path.
            nc.sync.dma_start(v_sbs[bh][:, 0:NG, :], v[bh][:, 0:NG, :])
        for bh in range(BH_PER_CORE):
            compute_group(bh, 0, qrt[bh][0], g0q[bh])
            emit_bc(bh, 0)

        for g in range(1, NG):
            for bh in range(BH_PER_CORE):
                qrt_g = qrtp.tile([P, NK, GW], bf16)
                qrt[bh].append(qrt_g)
                qt4 = load_group(bh, g)
                compute_group(bh, g, qrt_g, qt4)
                if g == 1:
                    nc.sync.dma_start(v_sbs[bh][:, NG:, :], v[bh][:, NG:, :])
            for bh in range(BH_PER_CORE):
                emit_bc(bh, g)
        pending_av()  # final group's last AV matmuls
        flush_drains()  # final group's output chunks


def build_nc():
    import concourse.bass as bass  # noqa: F401
    import concourse.mybir as mybir
    import concourse.tile as tile
    from concourse import bacc

    nc = bacc.Bacc(
        "TRN2",
        target_bir_lowering=False,
        debug=False,
        enable_asserts=False,
        num_devices=NCORES,
    )
    f32 = mybir.dt.float32
    bf16 = mybir.dt.bfloat16
    q = nc.dram_tensor(
        "q", [BH_PER_CORE, NG, P, NG, NDIM], bf16, kind="ExternalInput"
    ).ap()
    v = nc.dram_tensor(
        "v", [BH_PER_CORE, P, NT, DV], bf16, kind="ExternalInput"
    ).ap()
    cs = nc.dram_tensor(
        "cs", [NG, P, NG, 2, NH], bf16, kind="ExternalInput"
    ).ap()
    o = nc.dram_tensor("o", [BH_PER_CORE, T, DV], f32, kind="ExternalOutput").ap()

    with tile.TileContext(nc) as tc:
        _emit(tc, nc, (q, v, cs, o))
    nc.compile()
    return nc


def get_nc():
    if "nc" not in _NC_CACHE:
        _NC_CACHE["nc"] = build_nc()
    return _NC_CACHE["nc"]


def make_in_maps(Q, V, freqs):
    import ml_dtypes

    bf = ml_dtypes.bfloat16
    Q = np.asarray(Q, dtype=np.float32).reshape(B * H, T, NDIM)
    # even-pairs-first n permutation (see phase_a_pair)
    Q = np.concatenate([Q[..., 0::2], Q[..., 1::2]], axis=-1).astype(bf)
    # t-tile groups: [bh, 4, 128, 4, 512] so each q DMA reads 4KB/partition
    Q = Q.reshape(B * H, NG, NG, P, NDIM).transpose(0, 1, 3, 2, 4)
    V = np.asarray(V, dtype=np.float32).reshape(B * H, T, DV).astype(bf)
    # s-major: [bh, s, i, d] so V DMAs read 1KB+/partition
    V = V.reshape(B * H, NT, P, DV).transpose(0, 2, 1, 3)
    cosv, sinv = _host_tables(freqs)  # [T, NH] each
    cs = np.stack([cosv, sinv], axis=1).reshape(NG, NG, P, 2, NH)
    cs = np.ascontiguousarray(cs.transpose(0, 2, 1, 3, 4))  # [4,128,4,2,NH]
    in_maps = []
    for c in range(NCORES):
        in_maps.append(
            {
                "q": np.ascontiguousarray(Q[BH_PER_CORE * c : BH_PER_CORE * (c + 1)]),
                "v": np.ascontiguousarray(V[BH_PER_CORE * c : BH_PER_CORE * (c + 1)]),
                "cs": cs,
            }
        )
    return in_maps


def kernel(Q, V, freqs):
    global LAST_RESULTS
    from concourse.bass_utils import run_bass_kernel_spmd

    nc = get_nc()
    in_maps = make_in_maps(Q, V, freqs)
    res = run_bass_kernel_spmd(
        nc, in_maps, core_ids=list(range(NCORES)), trace=TRACE
    )
    LAST_RESULTS = res
    out = np.stack([r["o"] for r in res.results])  # [8, 2, T, DV]
    return out.reshape(B, H, T, DV).astype(np.float32)
